# revision 10
# baseline (speedup 1.0000x reference)
"""DilateAttention (3x3 kernel, dilation 2) Trainium2 Bass kernel.

Reference semantics (per batch b, head h, pixel n):
  logits[j] = sum_d q[d,n] * k[d, n + off_j] * 32**-0.5   (zero-padded)
  attn = softmax(logits)  (all 9 slots always participate; OOB -> logit 0)
  out[d, n] = sum_j attn[j] * v[d, n + off_j]

Strategy: data-parallel over batch B=8 across 8 cores. Per core the
[384, 56*56] problem runs in 3 head-groups of 128 channels (4 heads x
32 head_dim on partitions) x 7 row-chunks of 8 rows (448 pixels free).
Each chunk's q/k/v rows arrive as ONE packed DMA; the 9 dilated
neighbor reads are zero-copy strided window APs into that tile.

Broadcast-native dataflow (v2): the per-shift logit reduction matmul
uses an all-ones 32x32 block-diagonal stationary, so EVERY channel
partition of a head receives that head's logit j. One ACT exp per dy
(PSUM f32 -> SBUF bf16, scale folded in) then materializes the
attention weights already replicated across the 32 head_dim partitions
-- no separate PE broadcast and no PSUM->SBUF copies. The 9-way AV sum
runs as accumulating identity matmuls on the PE (fp32 PSUM), and the
denominators as 9 tiny [4,448] PE matmuls off the same eb tile.

Engines per chunk:
  - DVE:  one wide q*k product (dy 0/1) + the 3-dy wide attn*v product
  - GPSIMD: the dy=2 q*k product
  - PE:   9 logit matmuls + 9 avsum matmuls + 9 den matmuls
  - ACT:  3 exps (the only e materialization) + out/den PSUM->SBUF

The kernel returns the UNNORMALIZED weighted sum (bf16) plus the
denominators; the host performs the division (free), plus input
padding to 60x60, bf16 casts, per-chunk packing, and the final
transpose of [384, 3136] channel-major output to [56, 56, 384].
"""

import sys

sys.path.insert(0, "/opt/trn_rl_repo")

import numpy as np

import concourse.bass as bass
import concourse.mybir as mybir
from concourse import bacc, tile
from concourse.bass_utils import run_bass_kernel_spmd

B = 8
C = 384
H = W = 56
PAD = 2
HP = WP = 60
N = H * W
HG = 3            # head groups (128 channels each)
CH_ROWS = 8       # query rows per chunk
CH = CH_ROWS * W  # 448 pixels per chunk
NCH = H // CH_ROWS
SCALE = 32 ** -0.5

f32 = mybir.dt.float32
bf16 = mybir.dt.bfloat16

_CACHE = {}


KROWS = CH_ROWS + 4          # 12 padded k/v rows per chunk
QSEC = CH                    # 448
KSEC = KROWS * WP            # 720
XSEC = QSEC + 2 * KSEC       # 1888 elements per chunk per partition


def _win_ap(base, elem_off, dims):
    """Custom windowed AP over a 2D [128, XSEC] tile: partition dim from
    `base`, plus free dims given as [stride, count] pairs (elements)."""
    import bass_rust
    return bass_rust.AP(
        base.tensor, offset=base.offset + elem_off,
        ap=[list(base.ap[0])] + [list(d) for d in dims],
    )


def _build_nc():
    nc = bacc.Bacc("TRN2", target_bir_lowering=False)
    # Per (head-group, chunk) packed transfer: q rows then k rows then v
    # rows, contiguous per partition, so each chunk is ONE dma (one wait).
    x_d = nc.declare_dram_parameter("x", [HG, NCH, 128, XSEC], bf16,
                                    isOutput=False)
    cb_d = nc.declare_dram_parameter("cb", [128, 260], bf16, isOutput=False)
    o_d = nc.declare_dram_parameter("out", [C, N], bf16, isOutput=True)
    den_d = nc.declare_dram_parameter("den", [HG, NCH, 4, CH], bf16,
                                      isOutput=True)

    with tile.TileContext(nc) as tc:
        with (
            tc.tile_pool(name="const", bufs=1) as cpool,
            tc.tile_pool(name="inbuf", bufs=2) as ipool,
            tc.tile_pool(name="work", bufs=2) as wpool,
            tc.tile_pool(name="psA", bufs=2, space="PSUM") as psA,
            tc.tile_pool(name="psB", bufs=1, space="PSUM") as psB,
        ):
            # Host-prepared constant stationaries:
            #   Sones[32g+d', 32g+d] = 1  (head-block all-ones: logit bcast)
            #   Ident[p, p] = 1           (avsum identity)
            #   Wden[32g, g] = 1          (pick d'=0 row per head for den)
            cbuf = cpool.tile([128, 260], bf16)
            nc.sync.dma_start(out=cbuf[:], in_=cb_d[:])
            sones = cbuf[:, 0:128]
            ident = cbuf[:, 128:256]
            wden = cbuf[:, 256:260]

            WIN = [[2, 3], [WP, CH_ROWS], [1, W]]   # (dx, row, col) window

            for hg in range(HG):
                r0 = 128 * hg
                for ch in range(NCH):
                    y0 = ch * CH_ROWS
                    cin = ipool.tile([128, XSEC], bf16, tag="cin", bufs=3)
                    nc.sync.dma_start(out=cin[:], in_=x_d[hg, ch])
                    cb2 = cin[:]

                    # --- QK products: q broadcast over the 9 shifts.
                    # dy 0/1 in one wide DVE op, dy 2 on GPSIMD.
                    prod = wpool.tile([128, 3, 3, CH_ROWS, W], bf16,
                                      tag="prod", bufs=3)
                    qv3 = _win_ap(cb2, 0, [[0, 3], [W, CH_ROWS], [1, W]])
                    for dy in range(3):
                        kv3 = _win_ap(cb2, QSEC + 2 * dy * WP, WIN)
                        eng = nc.gpsimd if dy == 2 else nc.vector
                        eng.tensor_mul(prod[:, dy], qv3, kv3)

                    # --- logits, broadcast to all 32 head_dim partitions:
                    # ab3[dy][32g+d, dx, n] = sum_d' q[d',n]*k[d',n+off]
                    # --- exp: one ACT op per dy lands e (already broadcast)
                    # in SBUF bf16.  eb[32g+d, 3*dy+dx, n] = e_j[g, n]
                    eb = wpool.tile([128, 9, CH], bf16, tag="eb", bufs=2)
                    for dy in range(3):
                        ab3 = psA.tile([128, 3, 512], f32, tag="ab3",
                                       bufs=2)
                        for dx in range(3):
                            nc.tensor.matmul(
                                ab3[:, dx, 0:CH],
                                sones,
                                prod[:, dy, dx].rearrange(
                                    "p a b -> p (a b)"),
                                start=True, stop=True,
                            )
                        nc.scalar.activation(
                            eb[:, 3 * dy:3 * dy + 3],
                            ab3[:, :, 0:CH],
                            mybir.ActivationFunctionType.Exp,
                            scale=SCALE,
                        )

                    # --- osum (bank 0) and den (bank 1) share one PSUM
                    # tile so a single ACT copy drains both.
                    ops = psB.tile([128, 2, 512], f32, tag="ops", bufs=1)
                    osum = ops[:, 0, 0:CH]
                    den = ops[0:4, 1, 0:CH]
                    # --- denominators: den[g, n] = sum_j e_j[g, n] via 9
                    # accumulating [4, 448] matmuls picking row 32g.
                    for j in range(9):
                        nc.tensor.matmul(
                            den, wden, eb[:, j],
                            start=(j == 0), stop=(j == 8),
                        )

                    # --- AV: one wide DVE product (all 9 shifts), then the
                    # 9-way sum as accumulating identity matmuls (fp32).
                    avp = wpool.tile([128, 9, CH], bf16, tag="avp", bufs=2)
                    for dy in range(3):
                        vv3 = _win_ap(cb2, QSEC + KSEC + 2 * dy * WP, WIN)
                        nc.vector.tensor_mul(
                            avp[:, 3 * dy:3 * dy + 3].rearrange(
                                "p j (a b) -> p j a b", a=CH_ROWS),
                            eb[:, 3 * dy:3 * dy + 3].rearrange(
                                "p j (a b) -> p j a b", a=CH_ROWS),
                            vv3)
                    for j in range(9):
                        nc.tensor.matmul(
                            osum, ident, avp[:, j],
                            start=(j == 0), stop=(j == 8),
                        )
                    avs = wpool.tile([128, 2, CH], bf16, tag="avs")
                    nc.scalar.copy(avs[:], ops[:, :, 0:CH])
                    nc.sync.dma_start(
                        out=o_d[r0:r0 + 128, y0 * W:(y0 + CH_ROWS) * W],
                        in_=avs[:, 0],
                    )
                    nc.sync.dma_start(out=den_d[hg, ch], in_=avs[0:4, 1])
    nc.compile()
    return nc


def _get_nc():
    if "nc" not in _CACHE:
        _CACHE["nc"] = _build_nc()
    return _CACHE["nc"]


def _prep_inputs(q, k, v):
    """Full [8, 384, 56, 56] fp32 -> per-core bf16 input maps."""
    import ml_dtypes
    bfl = ml_dtypes.bfloat16
    kp = np.zeros((B, C, HP, WP), dtype=np.float32)
    vp = np.zeros((B, C, HP, WP), dtype=np.float32)
    kp[:, :, PAD:PAD + H, PAD:PAD + W] = k
    vp[:, :, PAD:PAD + H, PAD:PAD + W] = v
    cb = np.zeros((128, 260), dtype=np.float32)
    for g in range(4):
        cb[32 * g:32 * (g + 1), 32 * g:32 * (g + 1)] = 1.0   # Sones
        cb[32 * g, 256 + g] = 1.0                            # Wden
    cb[:, 128:256] = np.eye(128, dtype=np.float32)           # Ident
    cb = cb.astype(bfl)

    # Pack per (head-group, chunk): q rows [8,56], k rows [12,60], v rows
    # [12,60], flattened per channel partition -> one DMA per chunk.
    qr = q.reshape(B, HG, 128, H, W)
    kr = kp.reshape(B, HG, 128, HP, WP)
    vr = vp.reshape(B, HG, 128, HP, WP)
    x = np.empty((B, HG, NCH, 128, XSEC), dtype=np.float32)
    for ch in range(NCH):
        y0 = ch * CH_ROWS
        x[:, :, ch, :, 0:QSEC] = qr[:, :, :, y0:y0 + CH_ROWS, :].reshape(
            B, HG, 128, QSEC)
        x[:, :, ch, :, QSEC:QSEC + KSEC] = kr[
            :, :, :, y0:y0 + KROWS, :].reshape(B, HG, 128, KSEC)
        x[:, :, ch, :, QSEC + KSEC:XSEC] = vr[
            :, :, :, y0:y0 + KROWS, :].reshape(B, HG, 128, KSEC)
    x = x.astype(bfl)

    in_maps = []
    for b in range(B):
        in_maps.append({
            "x": np.ascontiguousarray(x[b]),
            "cb": cb,
        })
    return in_maps


def _run(q, k, v, trace=False):
    nc = _get_nc()
    in_maps = _prep_inputs(q, k, v)
    res = run_bass_kernel_spmd(nc, in_maps, list(range(B)), trace=trace)
    outs = []
    for b in range(B):
        o = np.asarray(res.results[b]["out"]).astype(np.float32)
        d = np.asarray(res.results[b]["den"], dtype=np.float32)
        o = o.reshape(HG, 4, 32, NCH, CH)
        o = o / d.transpose(0, 2, 1, 3)[:, :, None, :, :]
        outs.append(o.reshape(C, H, W).transpose(1, 2, 0))
    return np.stack(outs, axis=0), res


def kernel(q, k, v):
    out, _ = _run(np.asarray(q), np.asarray(k), np.asarray(v), trace=False)
    return out


def bench(q, k, v, iters=10):
    """Time repeated executions of the compiled NEFF on the 8 cores.

    Mirrors bass2jax.run_bass_via_pjrt's shard_map path but keeps the
    jitted executable and device-resident inputs, no donation, so each
    iteration is dispatch + hardware execution only.
    """
    import time

    import jax
    from jax.sharding import Mesh, PartitionSpec
    from jax.experimental.shard_map import shard_map

    from concourse import bass2jax
    from concourse.bass2jax import _bass_exec_p
    import concourse.mybir as mybir_

    nc = _get_nc()
    in_maps = _prep_inputs(np.asarray(q), np.asarray(k), np.asarray(v))
    bass2jax.install_neuronx_cc_hook()

    part_name = (nc.partition_id_tensor.name
                 if nc.partition_id_tensor else None)
    in_names, out_names, out_avals, zero_outs = [], [], [], []
    for alloc in nc.m.functions[0].allocations:
        if not isinstance(alloc, mybir_.MemoryLocationSet):
            continue
        name = alloc.memorylocations[0].name
        if alloc.kind == "ExternalInput":
            if name != part_name:
                in_names.append(name)
        elif alloc.kind == "ExternalOutput":
            out_names.append(name)
            dt_np = mybir_.dt.np(alloc.dtype)
            out_avals.append(
                jax.core.ShapedArray(tuple(alloc.tensor_shape), dt_np))
            zero_outs.append(
                np.zeros(tuple(alloc.tensor_shape), dt_np))
    n_params = len(in_names)
    all_names = in_names + out_names
    if part_name is not None:
        all_names = all_names + [part_name]

    def _body(*args):
        operands = list(args)
        if part_name is not None:
            operands.append(bass2jax.partition_id_tensor())
        outs = _bass_exec_p.bind(
            *operands,
            out_avals=tuple(out_avals),
            in_names=tuple(all_names),
            out_names=tuple(out_names),
            lowering_input_output_aliases=(),
            sim_require_finite=True,
            sim_require_nnan=True,
            nc=nc,
        )
        return tuple(outs)

    devices = jax.devices()[:B]
    mesh = Mesh(np.asarray(devices), ("core",))
    nin = n_params + len(out_names)
    sharded = jax.jit(
        shard_map(
            _body, mesh=mesh,
            in_specs=(PartitionSpec("core"),) * nin,
            out_specs=(PartitionSpec("core"),) * len(out_names),
            check_rep=False,
        ),
        keep_unused=True,
    )
    concat_in = [
        np.concatenate([np.asarray(in_maps[c][nm]) for c in range(B)], axis=0)
        for nm in in_names
    ]
    concat_zero = [
        np.zeros((B * z.shape[0], *z.shape[1:]), z.dtype) for z in zero_outs
    ]
    args = [jax.device_put(a) for a in concat_in + concat_zero]
    # warmup (compile)
    out = sharded(*args)
    jax.block_until_ready(out)
    times = []
    for _ in range(iters):
        t0 = time.perf_counter()
        out = sharded(*args)
        jax.block_until_ready(out)
        times.append(time.perf_counter() - t0)
    oi = out_names.index("out")
    di = out_names.index("den")
    o_all = np.asarray(out[oi]).astype(np.float32).reshape(B, C, N)
    d_all = np.asarray(out[di]).astype(np.float32).reshape(B, HG, NCH, 4, CH)
    outs = []
    for b in range(B):
        o = o_all[b].reshape(HG, 4, 32, NCH, CH)
        o = o / d_all[b].transpose(0, 2, 1, 3)[:, :, None, :, :]
        outs.append(o.reshape(C, H, W).transpose(1, 2, 0))
    return times, np.stack(outs, axis=0)


# revision 11
# speedup vs baseline: 1.1772x; 1.1772x over previous
"""DilateAttention (3x3 kernel, dilation 2) Trainium2 Bass kernel.

Reference semantics (per batch b, head h, pixel n):
  logits[j] = sum_d q[d,n] * k[d, n + off_j] * 32**-0.5   (zero-padded)
  attn = softmax(logits)  (all 9 slots always participate; OOB -> logit 0)
  out[d, n] = sum_j attn[j] * v[d, n + off_j]

Strategy: data-parallel over batch B=8 across 8 cores. Per core the
[384, 56*56] problem runs in 3 head-groups of 128 channels (4 heads x
32 head_dim on partitions) x 7 row-chunks of 8 rows (448 pixels free).
Each chunk's q/k/v rows arrive as ONE packed DMA; the 9 dilated
neighbor reads are zero-copy strided window APs into that tile.

Broadcast-native dataflow (v2): the per-shift logit reduction matmul
uses an all-ones 32x32 block-diagonal stationary, so EVERY channel
partition of a head receives that head's logit j. One ACT exp per dy
(PSUM f32 -> SBUF bf16, scale folded in) then materializes the
attention weights already replicated across the 32 head_dim partitions
-- no separate PE broadcast and no PSUM->SBUF copies. The 9-way AV sum
runs as accumulating identity matmuls on the PE (fp32 PSUM), and the
denominators as 9 tiny [4,448] PE matmuls off the same eb tile.

Engines per chunk:
  - DVE:  one wide q*k product (dy 0/1) + the 3-dy wide attn*v product
  - GPSIMD: the dy=2 q*k product
  - PE:   9 logit matmuls + 9 avsum matmuls + 9 den matmuls
  - ACT:  3 exps (the only e materialization) + out/den PSUM->SBUF

The kernel returns the UNNORMALIZED weighted sum (bf16) plus the
denominators; the host performs the division (free), plus input
padding to 60x60, bf16 casts, per-chunk packing, and the final
transpose of [384, 3136] channel-major output to [56, 56, 384].
"""

import sys

sys.path.insert(0, "/opt/trn_rl_repo")

import numpy as np

import concourse.bass as bass
import concourse.mybir as mybir
from concourse import bacc, tile
from concourse.bass_utils import run_bass_kernel_spmd

B = 8
C = 384
H = W = 56
PAD = 2
HP = WP = 60
N = H * W
HG = 3            # head groups (128 channels each)
CH_ROWS = 8       # query rows per chunk
CH = CH_ROWS * W  # 448 pixels per chunk
NCH = H // CH_ROWS
SCALE = 32 ** -0.5

f32 = mybir.dt.float32
bf16 = mybir.dt.bfloat16

_CACHE = {}


KROWS = CH_ROWS + 4          # 12 padded k/v rows per chunk
QSEC = CH                    # 448
KSEC = KROWS * WP            # 720
XSEC = QSEC + 2 * KSEC       # 1888 elements per chunk per partition


def _win_ap(base, elem_off, dims):
    """Custom windowed AP over a 2D [128, XSEC] tile: partition dim from
    `base`, plus free dims given as [stride, count] pairs (elements)."""
    import bass_rust
    return bass_rust.AP(
        base.tensor, offset=base.offset + elem_off,
        ap=[list(base.ap[0])] + [list(d) for d in dims],
    )


def _build_nc():
    nc = bacc.Bacc("TRN2", target_bir_lowering=False)
    # Per (head-group, chunk) packed transfer: q rows then k rows then v
    # rows, contiguous per partition, so each chunk is ONE dma (one wait).
    x_d = nc.declare_dram_parameter("x", [HG, NCH, 128, XSEC], bf16,
                                    isOutput=False)
    cb_d = nc.declare_dram_parameter("cb", [128, 260], bf16, isOutput=False)
    o_d = nc.declare_dram_parameter("out", [C, N], bf16, isOutput=True)
    den_d = nc.declare_dram_parameter("den", [HG, NCH, 4, CH], bf16,
                                      isOutput=True)

    with tile.TileContext(nc) as tc:
        with (
            tc.tile_pool(name="const", bufs=1) as cpool,
            tc.tile_pool(name="inbuf", bufs=2) as ipool,
            tc.tile_pool(name="work", bufs=2) as wpool,
            tc.tile_pool(name="psA", bufs=2, space="PSUM") as psA,
            tc.tile_pool(name="psB", bufs=1, space="PSUM") as psB,
        ):
            # Host-prepared constant stationaries:
            #   Sones[32g+d', 32g+d] = 1  (head-block all-ones: logit bcast)
            #   Ident[p, p] = 1           (avsum identity)
            #   Wden[32g, g] = 1          (pick d'=0 row per head for den)
            cbuf = cpool.tile([128, 260], bf16)
            nc.sync.dma_start(out=cbuf[:], in_=cb_d[:])
            sones = cbuf[:, 0:128]
            ident = cbuf[:, 128:256]
            wden = cbuf[:, 256:260]

            WIN = [[2, 3], [WP, CH_ROWS], [1, W]]   # (dx, row, col) window

            def stage_a(hg, ch):
                """Load + QK products + broadcast logits + exp."""
                cin = ipool.tile([128, XSEC], bf16, tag="cin", bufs=3)
                nc.sync.dma_start(out=cin[:], in_=x_d[hg, ch])
                cb2 = cin[:]
                prod = wpool.tile([128, 3, 3, CH_ROWS, W], bf16,
                                  tag="prod", bufs=2)
                qv3 = _win_ap(cb2, 0, [[0, 3], [W, CH_ROWS], [1, W]])
                for dy in range(3):
                    kv3 = _win_ap(cb2, QSEC + 2 * dy * WP, WIN)
                    eng = nc.gpsimd if dy == 2 else nc.vector
                    eng.tensor_mul(prod[:, dy], qv3, kv3)
                # eb[32g+d, 3*dy+dx, n] = e_j[g, n] (already broadcast over
                # the head_dim partitions by the all-ones block stationary)
                eb = wpool.tile([128, 9, CH], bf16, tag="eb", bufs=2)
                for dy in range(3):
                    ab3 = psA.tile([128, 3, 512], f32, tag="ab3", bufs=2)
                    for dx in range(3):
                        nc.tensor.matmul(
                            ab3[:, dx, 0:CH],
                            sones,
                            prod[:, dy, dx].rearrange("p a b -> p (a b)"),
                            start=True, stop=True,
                        )
                    nc.scalar.activation(
                        eb[:, 3 * dy:3 * dy + 3],
                        ab3[:, :, 0:CH],
                        mybir.ActivationFunctionType.Exp,
                        scale=SCALE,
                    )
                return cin, eb

            def stage_b(hg, ch, cin, eb):
                """den + AV products + PE 9-way sums + drain + store."""
                r0 = 128 * hg
                y0 = ch * CH_ROWS
                cb2 = cin[:]
                # osum (bank 0) and den (bank 1) share one PSUM tile so a
                # single ACT copy drains both.
                ops = psB.tile([128, 2, 512], f32, tag="ops", bufs=1)
                osum = ops[:, 0, 0:CH]
                den = ops[0:4, 1, 0:CH]
                for j in range(9):
                    nc.tensor.matmul(
                        den, wden, eb[:, j],
                        start=(j == 0), stop=(j == 8),
                    )
                avp = wpool.tile([128, 9, CH], bf16, tag="avp", bufs=2)
                for dy in range(3):
                    vv3 = _win_ap(cb2, QSEC + KSEC + 2 * dy * WP, WIN)
                    nc.vector.tensor_mul(
                        avp[:, 3 * dy:3 * dy + 3].rearrange(
                            "p j (a b) -> p j a b", a=CH_ROWS),
                        eb[:, 3 * dy:3 * dy + 3].rearrange(
                            "p j (a b) -> p j a b", a=CH_ROWS),
                        vv3)
                for j in range(9):
                    nc.tensor.matmul(
                        osum, ident, avp[:, j],
                        start=(j == 0), stop=(j == 8),
                    )
                avs = wpool.tile([128, 2, CH], bf16, tag="avs")
                nc.scalar.copy(avs[:], ops[:, :, 0:CH])
                nc.sync.dma_start(
                    out=o_d[r0:r0 + 128, y0 * W:(y0 + CH_ROWS) * W],
                    in_=avs[:, 0],
                )
                nc.sync.dma_start(out=den_d[hg, ch], in_=avs[0:4, 1])

            # Software pipeline: stage A of chunk i+1 is issued before
            # stage B of chunk i so the in-order DVE/PE/ACT streams always
            # have cross-chunk work to hide each other's latencies.
            chunks = [(hg, ch) for hg in range(HG) for ch in range(NCH)]
            pending = None
            for hg, ch in chunks:
                handles = stage_a(hg, ch)
                if pending is not None:
                    stage_b(*pending)
                pending = (hg, ch) + (handles[0], handles[1])
            stage_b(*pending)
    nc.compile()
    return nc


def _get_nc():
    if "nc" not in _CACHE:
        _CACHE["nc"] = _build_nc()
    return _CACHE["nc"]


def _prep_inputs(q, k, v):
    """Full [8, 384, 56, 56] fp32 -> per-core bf16 input maps."""
    import ml_dtypes
    bfl = ml_dtypes.bfloat16
    kp = np.zeros((B, C, HP, WP), dtype=np.float32)
    vp = np.zeros((B, C, HP, WP), dtype=np.float32)
    kp[:, :, PAD:PAD + H, PAD:PAD + W] = k
    vp[:, :, PAD:PAD + H, PAD:PAD + W] = v
    cb = np.zeros((128, 260), dtype=np.float32)
    for g in range(4):
        cb[32 * g:32 * (g + 1), 32 * g:32 * (g + 1)] = 1.0   # Sones
        cb[32 * g, 256 + g] = 1.0                            # Wden
    cb[:, 128:256] = np.eye(128, dtype=np.float32)           # Ident
    cb = cb.astype(bfl)

    # Pack per (head-group, chunk): q rows [8,56], k rows [12,60], v rows
    # [12,60], flattened per channel partition -> one DMA per chunk.
    qr = q.reshape(B, HG, 128, H, W)
    kr = kp.reshape(B, HG, 128, HP, WP)
    vr = vp.reshape(B, HG, 128, HP, WP)
    x = np.empty((B, HG, NCH, 128, XSEC), dtype=np.float32)
    for ch in range(NCH):
        y0 = ch * CH_ROWS
        x[:, :, ch, :, 0:QSEC] = qr[:, :, :, y0:y0 + CH_ROWS, :].reshape(
            B, HG, 128, QSEC)
        x[:, :, ch, :, QSEC:QSEC + KSEC] = kr[
            :, :, :, y0:y0 + KROWS, :].reshape(B, HG, 128, KSEC)
        x[:, :, ch, :, QSEC + KSEC:XSEC] = vr[
            :, :, :, y0:y0 + KROWS, :].reshape(B, HG, 128, KSEC)
    x = x.astype(bfl)

    in_maps = []
    for b in range(B):
        in_maps.append({
            "x": np.ascontiguousarray(x[b]),
            "cb": cb,
        })
    return in_maps


def _run(q, k, v, trace=False):
    nc = _get_nc()
    in_maps = _prep_inputs(q, k, v)
    res = run_bass_kernel_spmd(nc, in_maps, list(range(B)), trace=trace)
    outs = []
    for b in range(B):
        o = np.asarray(res.results[b]["out"]).astype(np.float32)
        d = np.asarray(res.results[b]["den"], dtype=np.float32)
        o = o.reshape(HG, 4, 32, NCH, CH)
        o = o / d.transpose(0, 2, 1, 3)[:, :, None, :, :]
        outs.append(o.reshape(C, H, W).transpose(1, 2, 0))
    return np.stack(outs, axis=0), res


def kernel(q, k, v):
    out, _ = _run(np.asarray(q), np.asarray(k), np.asarray(v), trace=False)
    return out


def bench(q, k, v, iters=10):
    """Time repeated executions of the compiled NEFF on the 8 cores.

    Mirrors bass2jax.run_bass_via_pjrt's shard_map path but keeps the
    jitted executable and device-resident inputs, no donation, so each
    iteration is dispatch + hardware execution only.
    """
    import time

    import jax
    from jax.sharding import Mesh, PartitionSpec
    from jax.experimental.shard_map import shard_map

    from concourse import bass2jax
    from concourse.bass2jax import _bass_exec_p
    import concourse.mybir as mybir_

    nc = _get_nc()
    in_maps = _prep_inputs(np.asarray(q), np.asarray(k), np.asarray(v))
    bass2jax.install_neuronx_cc_hook()

    part_name = (nc.partition_id_tensor.name
                 if nc.partition_id_tensor else None)
    in_names, out_names, out_avals, zero_outs = [], [], [], []
    for alloc in nc.m.functions[0].allocations:
        if not isinstance(alloc, mybir_.MemoryLocationSet):
            continue
        name = alloc.memorylocations[0].name
        if alloc.kind == "ExternalInput":
            if name != part_name:
                in_names.append(name)
        elif alloc.kind == "ExternalOutput":
            out_names.append(name)
            dt_np = mybir_.dt.np(alloc.dtype)
            out_avals.append(
                jax.core.ShapedArray(tuple(alloc.tensor_shape), dt_np))
            zero_outs.append(
                np.zeros(tuple(alloc.tensor_shape), dt_np))
    n_params = len(in_names)
    all_names = in_names + out_names
    if part_name is not None:
        all_names = all_names + [part_name]

    def _body(*args):
        operands = list(args)
        if part_name is not None:
            operands.append(bass2jax.partition_id_tensor())
        outs = _bass_exec_p.bind(
            *operands,
            out_avals=tuple(out_avals),
            in_names=tuple(all_names),
            out_names=tuple(out_names),
            lowering_input_output_aliases=(),
            sim_require_finite=True,
            sim_require_nnan=True,
            nc=nc,
        )
        return tuple(outs)

    devices = jax.devices()[:B]
    mesh = Mesh(np.asarray(devices), ("core",))
    nin = n_params + len(out_names)
    sharded = jax.jit(
        shard_map(
            _body, mesh=mesh,
            in_specs=(PartitionSpec("core"),) * nin,
            out_specs=(PartitionSpec("core"),) * len(out_names),
            check_rep=False,
        ),
        keep_unused=True,
    )
    concat_in = [
        np.concatenate([np.asarray(in_maps[c][nm]) for c in range(B)], axis=0)
        for nm in in_names
    ]
    concat_zero = [
        np.zeros((B * z.shape[0], *z.shape[1:]), z.dtype) for z in zero_outs
    ]
    args = [jax.device_put(a) for a in concat_in + concat_zero]
    # warmup (compile)
    out = sharded(*args)
    jax.block_until_ready(out)
    times = []
    for _ in range(iters):
        t0 = time.perf_counter()
        out = sharded(*args)
        jax.block_until_ready(out)
        times.append(time.perf_counter() - t0)
    oi = out_names.index("out")
    di = out_names.index("den")
    o_all = np.asarray(out[oi]).astype(np.float32).reshape(B, C, N)
    d_all = np.asarray(out[di]).astype(np.float32).reshape(B, HG, NCH, 4, CH)
    outs = []
    for b in range(B):
        o = o_all[b].reshape(HG, 4, 32, NCH, CH)
        o = o / d_all[b].transpose(0, 2, 1, 3)[:, :, None, :, :]
        outs.append(o.reshape(C, H, W).transpose(1, 2, 0))
    return times, np.stack(outs, axis=0)


# revision 12
# speedup vs baseline: 1.2448x; 1.0574x over previous
"""DilateAttention (3x3 kernel, dilation 2) Trainium2 Bass kernel.

Reference semantics (per batch b, head h, pixel n):
  logits[j] = sum_d q[d,n] * k[d, n + off_j] * 32**-0.5   (zero-padded)
  attn = softmax(logits)  (all 9 slots always participate; OOB -> logit 0)
  out[d, n] = sum_j attn[j] * v[d, n + off_j]

Strategy: data-parallel over batch B=8 across 8 cores. Per core the
[384, 56*56] problem runs in 3 head-groups of 128 channels (4 heads x
32 head_dim on partitions) x 7 row-chunks of 8 rows (448 pixels free).
Each chunk's q/k/v rows arrive as ONE packed DMA; the 9 dilated
neighbor reads are zero-copy strided window APs into that tile.

Broadcast-native dataflow (v2): the per-shift logit reduction matmul
uses an all-ones 32x32 block-diagonal stationary, so EVERY channel
partition of a head receives that head's logit j. One ACT exp per dy
(PSUM f32 -> SBUF bf16, scale folded in) then materializes the
attention weights already replicated across the 32 head_dim partitions
-- no separate PE broadcast and no PSUM->SBUF copies. The 9-way AV sum
runs as accumulating identity matmuls on the PE (fp32 PSUM), and the
denominators as 9 tiny [4,448] PE matmuls off the same eb tile.

Engines per chunk:
  - DVE:  one wide q*k product (dy 0/1) + the 3-dy wide attn*v product
  - GPSIMD: the dy=2 q*k product
  - PE:   9 logit matmuls + 9 avsum matmuls + 9 den matmuls
  - ACT:  3 exps (the only e materialization) + out/den PSUM->SBUF

The kernel returns the UNNORMALIZED weighted sum (bf16) plus the
denominators; the host performs the division (free), plus input
padding to 60x60, bf16 casts, per-chunk packing, and the final
transpose of [384, 3136] channel-major output to [56, 56, 384].
"""

import sys

sys.path.insert(0, "/opt/trn_rl_repo")

import numpy as np

import concourse.bass as bass
import concourse.mybir as mybir
from concourse import bacc, tile
from concourse.bass_utils import run_bass_kernel_spmd

B = 8
C = 384
H = W = 56
PAD = 2
HP = WP = 60
N = H * W
HG = 3            # head groups (128 channels each)
CH_ROWS = 8       # query rows per chunk
CH = CH_ROWS * W  # 448 pixels per chunk
NCH = H // CH_ROWS
SCALE = 32 ** -0.5

f32 = mybir.dt.float32
bf16 = mybir.dt.bfloat16

_CACHE = {}


KROWS = CH_ROWS + 4          # 12 padded k/v rows per chunk
QSEC = CH                    # 448
KSEC = KROWS * WP            # 720
XSEC = QSEC + 2 * KSEC       # 1888 elements per chunk per partition


def _win_ap(base, elem_off, dims):
    """Custom windowed AP over a 2D [128, XSEC] tile: partition dim from
    `base`, plus free dims given as [stride, count] pairs (elements)."""
    import bass_rust
    return bass_rust.AP(
        base.tensor, offset=base.offset + elem_off,
        ap=[list(base.ap[0])] + [list(d) for d in dims],
    )


def _build_nc():
    nc = bacc.Bacc("TRN2", target_bir_lowering=False)
    # Per (head-group, chunk) packed transfer: q rows then k rows then v
    # rows, contiguous per partition, so each chunk is ONE dma (one wait).
    x_d = nc.declare_dram_parameter("x", [HG, NCH, 128, XSEC], bf16,
                                    isOutput=False)
    cb_d = nc.declare_dram_parameter("cb", [128, 260], bf16, isOutput=False)
    o_d = nc.declare_dram_parameter("out", [C, N], bf16, isOutput=True)
    den_d = nc.declare_dram_parameter("den", [HG, NCH, 4, CH], bf16,
                                      isOutput=True)

    with tile.TileContext(nc) as tc:
        with (
            tc.tile_pool(name="const", bufs=1) as cpool,
            tc.tile_pool(name="inbuf", bufs=2) as ipool,
            tc.tile_pool(name="work", bufs=2) as wpool,
            tc.tile_pool(name="psA", bufs=2, space="PSUM") as psA,
            tc.tile_pool(name="psB", bufs=1, space="PSUM") as psB,
        ):
            # Host-prepared constant stationaries:
            #   Sones[32g+d', 32g+d] = 1  (head-block all-ones: logit bcast)
            #   Ident[p, p] = 1           (avsum identity)
            #   Wden[32g, g] = 1          (pick d'=0 row per head for den)
            cbuf = cpool.tile([128, 260], bf16)
            nc.sync.dma_start(out=cbuf[:], in_=cb_d[:])
            sones = cbuf[:, 0:128]
            ident = cbuf[:, 128:256]
            wden = cbuf[:, 256:260]

            WIN = [[2, 3], [WP, CH_ROWS], [1, W]]   # (dx, row, col) window

            def stage_a(hg, ch):
                """Load + QK products + broadcast logits + exp."""
                cin = ipool.tile([128, XSEC], bf16, tag="cin", bufs=3)
                nc.sync.dma_start(out=cin[:], in_=x_d[hg, ch])
                cb2 = cin[:]
                prod = wpool.tile([128, 3, 3, CH_ROWS, W], bf16,
                                  tag="prod", bufs=2)
                qv3 = _win_ap(cb2, 0, [[0, 3], [W, CH_ROWS], [1, W]])
                for dy in range(3):
                    kv3 = _win_ap(cb2, QSEC + 2 * dy * WP, WIN)
                    eng = nc.gpsimd if dy == 2 else nc.vector
                    eng.tensor_mul(prod[:, dy], qv3, kv3)
                # eb[32g+d, 3*dy+dx, n] = e_j[g, n] (already broadcast over
                # the head_dim partitions by the all-ones block stationary)
                eb = wpool.tile([128, 9, CH], bf16, tag="eb", bufs=2)
                for dy in range(3):
                    ab3 = psA.tile([128, 3, 512], f32, tag="ab3", bufs=2)
                    for dx in range(3):
                        nc.tensor.matmul(
                            ab3[:, dx, 0:CH],
                            sones,
                            prod[:, dy, dx].rearrange("p a b -> p (a b)"),
                            start=True, stop=True,
                        )
                    nc.scalar.activation(
                        eb[:, 3 * dy:3 * dy + 3],
                        ab3[:, :, 0:CH],
                        mybir.ActivationFunctionType.Exp,
                        scale=SCALE,
                    )
                return cin, eb

            def stage_b(hg, ch, cin, eb):
                """den + AV products + PE 9-way sums + drain + store."""
                r0 = 128 * hg
                y0 = ch * CH_ROWS
                cb2 = cin[:]
                # osum (bank 0) and den (bank 1) share one PSUM tile so a
                # single ACT copy drains both.
                ops = psB.tile([128, 2, 512], f32, tag="ops", bufs=1)
                osum = ops[:, 0, 0:CH]
                den = ops[0:4, 1, 0:CH]
                # DVE pre-adds e-slot pairs so den needs only 5 matmuls.
                es = wpool.tile([128, 4, CH], bf16, tag="es")
                nc.vector.tensor_add(es[:], eb[:, 0:4], eb[:, 4:8])
                for j in range(5):
                    mv = es[:, j] if j < 4 else eb[:, 8]
                    nc.tensor.matmul(
                        den, wden, mv,
                        start=(j == 0), stop=(j == 4),
                    )
                avp = wpool.tile([128, 9, CH], bf16, tag="avp", bufs=2)
                for dy in range(3):
                    vv3 = _win_ap(cb2, QSEC + KSEC + 2 * dy * WP, WIN)
                    nc.vector.tensor_mul(
                        avp[:, 3 * dy:3 * dy + 3].rearrange(
                            "p j (a b) -> p j a b", a=CH_ROWS),
                        eb[:, 3 * dy:3 * dy + 3].rearrange(
                            "p j (a b) -> p j a b", a=CH_ROWS),
                        vv3)
                for j in range(9):
                    nc.tensor.matmul(
                        osum, ident, avp[:, j],
                        start=(j == 0), stop=(j == 8),
                    )
                avs = wpool.tile([128, 2, CH], bf16, tag="avs")
                nc.scalar.copy(avs[:], ops[:, :, 0:CH])
                nc.sync.dma_start(
                    out=o_d[r0:r0 + 128, y0 * W:(y0 + CH_ROWS) * W],
                    in_=avs[:, 0],
                )
                nc.sync.dma_start(out=den_d[hg, ch], in_=avs[0:4, 1])

            # Software pipeline: stage A of chunk i+1 is issued before
            # stage B of chunk i so the in-order DVE/PE/ACT streams always
            # have cross-chunk work to hide each other's latencies.
            chunks = [(hg, ch) for hg in range(HG) for ch in range(NCH)]
            pending = None
            for hg, ch in chunks:
                handles = stage_a(hg, ch)
                if pending is not None:
                    stage_b(*pending)
                pending = (hg, ch) + (handles[0], handles[1])
            stage_b(*pending)
    nc.compile()
    return nc


def _get_nc():
    if "nc" not in _CACHE:
        _CACHE["nc"] = _build_nc()
    return _CACHE["nc"]


def _prep_inputs(q, k, v):
    """Full [8, 384, 56, 56] fp32 -> per-core bf16 input maps."""
    import ml_dtypes
    bfl = ml_dtypes.bfloat16
    kp = np.zeros((B, C, HP, WP), dtype=np.float32)
    vp = np.zeros((B, C, HP, WP), dtype=np.float32)
    kp[:, :, PAD:PAD + H, PAD:PAD + W] = k
    vp[:, :, PAD:PAD + H, PAD:PAD + W] = v
    cb = np.zeros((128, 260), dtype=np.float32)
    for g in range(4):
        cb[32 * g:32 * (g + 1), 32 * g:32 * (g + 1)] = 1.0   # Sones
        cb[32 * g, 256 + g] = 1.0                            # Wden
    cb[:, 128:256] = np.eye(128, dtype=np.float32)           # Ident
    cb = cb.astype(bfl)

    # Pack per (head-group, chunk): q rows [8,56], k rows [12,60], v rows
    # [12,60], flattened per channel partition -> one DMA per chunk.
    qr = q.reshape(B, HG, 128, H, W)
    kr = kp.reshape(B, HG, 128, HP, WP)
    vr = vp.reshape(B, HG, 128, HP, WP)
    x = np.empty((B, HG, NCH, 128, XSEC), dtype=np.float32)
    for ch in range(NCH):
        y0 = ch * CH_ROWS
        x[:, :, ch, :, 0:QSEC] = qr[:, :, :, y0:y0 + CH_ROWS, :].reshape(
            B, HG, 128, QSEC)
        x[:, :, ch, :, QSEC:QSEC + KSEC] = kr[
            :, :, :, y0:y0 + KROWS, :].reshape(B, HG, 128, KSEC)
        x[:, :, ch, :, QSEC + KSEC:XSEC] = vr[
            :, :, :, y0:y0 + KROWS, :].reshape(B, HG, 128, KSEC)
    x = x.astype(bfl)

    in_maps = []
    for b in range(B):
        in_maps.append({
            "x": np.ascontiguousarray(x[b]),
            "cb": cb,
        })
    return in_maps


def _run(q, k, v, trace=False):
    nc = _get_nc()
    in_maps = _prep_inputs(q, k, v)
    res = run_bass_kernel_spmd(nc, in_maps, list(range(B)), trace=trace)
    outs = []
    for b in range(B):
        o = np.asarray(res.results[b]["out"]).astype(np.float32)
        d = np.asarray(res.results[b]["den"], dtype=np.float32)
        o = o.reshape(HG, 4, 32, NCH, CH)
        o = o / d.transpose(0, 2, 1, 3)[:, :, None, :, :]
        outs.append(o.reshape(C, H, W).transpose(1, 2, 0))
    return np.stack(outs, axis=0), res


def kernel(q, k, v):
    out, _ = _run(np.asarray(q), np.asarray(k), np.asarray(v), trace=False)
    return out


def bench(q, k, v, iters=10):
    """Time repeated executions of the compiled NEFF on the 8 cores.

    Mirrors bass2jax.run_bass_via_pjrt's shard_map path but keeps the
    jitted executable and device-resident inputs, no donation, so each
    iteration is dispatch + hardware execution only.
    """
    import time

    import jax
    from jax.sharding import Mesh, PartitionSpec
    from jax.experimental.shard_map import shard_map

    from concourse import bass2jax
    from concourse.bass2jax import _bass_exec_p
    import concourse.mybir as mybir_

    nc = _get_nc()
    in_maps = _prep_inputs(np.asarray(q), np.asarray(k), np.asarray(v))
    bass2jax.install_neuronx_cc_hook()

    part_name = (nc.partition_id_tensor.name
                 if nc.partition_id_tensor else None)
    in_names, out_names, out_avals, zero_outs = [], [], [], []
    for alloc in nc.m.functions[0].allocations:
        if not isinstance(alloc, mybir_.MemoryLocationSet):
            continue
        name = alloc.memorylocations[0].name
        if alloc.kind == "ExternalInput":
            if name != part_name:
                in_names.append(name)
        elif alloc.kind == "ExternalOutput":
            out_names.append(name)
            dt_np = mybir_.dt.np(alloc.dtype)
            out_avals.append(
                jax.core.ShapedArray(tuple(alloc.tensor_shape), dt_np))
            zero_outs.append(
                np.zeros(tuple(alloc.tensor_shape), dt_np))
    n_params = len(in_names)
    all_names = in_names + out_names
    if part_name is not None:
        all_names = all_names + [part_name]

    def _body(*args):
        operands = list(args)
        if part_name is not None:
            operands.append(bass2jax.partition_id_tensor())
        outs = _bass_exec_p.bind(
            *operands,
            out_avals=tuple(out_avals),
            in_names=tuple(all_names),
            out_names=tuple(out_names),
            lowering_input_output_aliases=(),
            sim_require_finite=True,
            sim_require_nnan=True,
            nc=nc,
        )
        return tuple(outs)

    devices = jax.devices()[:B]
    mesh = Mesh(np.asarray(devices), ("core",))
    nin = n_params + len(out_names)
    sharded = jax.jit(
        shard_map(
            _body, mesh=mesh,
            in_specs=(PartitionSpec("core"),) * nin,
            out_specs=(PartitionSpec("core"),) * len(out_names),
            check_rep=False,
        ),
        keep_unused=True,
    )
    concat_in = [
        np.concatenate([np.asarray(in_maps[c][nm]) for c in range(B)], axis=0)
        for nm in in_names
    ]
    concat_zero = [
        np.zeros((B * z.shape[0], *z.shape[1:]), z.dtype) for z in zero_outs
    ]
    args = [jax.device_put(a) for a in concat_in + concat_zero]
    # warmup (compile)
    out = sharded(*args)
    jax.block_until_ready(out)
    times = []
    for _ in range(iters):
        t0 = time.perf_counter()
        out = sharded(*args)
        jax.block_until_ready(out)
        times.append(time.perf_counter() - t0)
    oi = out_names.index("out")
    di = out_names.index("den")
    o_all = np.asarray(out[oi]).astype(np.float32).reshape(B, C, N)
    d_all = np.asarray(out[di]).astype(np.float32).reshape(B, HG, NCH, 4, CH)
    outs = []
    for b in range(B):
        o = o_all[b].reshape(HG, 4, 32, NCH, CH)
        o = o / d_all[b].transpose(0, 2, 1, 3)[:, :, None, :, :]
        outs.append(o.reshape(C, H, W).transpose(1, 2, 0))
    return times, np.stack(outs, axis=0)
